# revision 27
# baseline (speedup 1.0000x reference)
"""Trainium2 Bass kernel for nn_AdjacencyMatrix — row-parallel middle steps.

c_{k+1} = W^T c_k chain.  Step 1 is column-style over W[0:1024, cols_d]
(local chunk out); steps 2..3 are ROW-style over the resident row-shard
Wr_d = W[1024d:1024(d+1), :]: each consumes only the LOCAL c-chunk, so
the first collective (an 8-way ReduceScatter of the [1,8192] partials)
is not needed until after step 2 has already run — the fixed ~50us NRT
gang-barrier + first-collective cold start overlap with compute and the
W stream instead of gating them.  Step 4 uses the local W4 row-slice
(diag folded) and each core outputs its 256-float partial; the host sums
the 8 partials while gathering.
"""

import ml_dtypes
import numpy as np

import concourse.bass as bass
import concourse.mybir as mybir
from concourse import bacc, tile
from concourse.bass_utils import run_bass_kernel_spmd

N = 8192
IN_N = 1024
OUT_N = 256
NCORES = 8
CP = N // NCORES
KT = N // 128
D0 = N - OUT_N
NBLK = 4          # W chase blocks: columns [2048b, 2048(b+1)) of the row shard
CPB = N // NBLK   # 2048 columns per block

F32 = mybir.dt.float32
BF16 = mybir.dt.bfloat16
RG = [list(range(NCORES))]

_cache: dict = {}


def _row_waves(nc, pP, u_sb, wk, part_sb):
    """Row-style matvec: partial[c] = sum_j Wr[128j.., c]^T u[:, j].
    Two passes of 4096 output cols (psum budget); 4 col-groups x 1024."""
    for p in range(2):
        for j in range(8):
            for g in range(4):
                for sub in range(2):
                    C = 4096 * p + 1024 * g + 512 * sub
                    b = C // CPB
                    o = C - CPB * b
                    nc.tensor.matmul(
                        pP[32 * g:32 * g + 1, 512 * sub:512 * (sub + 1)],
                        lhsT=u_sb[:, j:j + 1],
                        rhs=wk[:, (b * 8 + j) * CPB + o:(b * 8 + j) * CPB + o + 512],
                        start=(j == 0),
                        stop=(j == 7),
                        tile_position=(0, 32 * g),
                    )
        for g in range(4):
            eng = nc.vector.tensor_copy if g % 2 == 0 else nc.scalar.copy
            eng(out=part_sb[0:1, 4096 * p + 1024 * g:4096 * p + 1024 * (g + 1)],
                in_=pP[32 * g:32 * g + 1, 0:1024])


def _chunk_to_u(nc, csb, u_sb, pTf, idtf):
    """[1,1024] f32 chunk -> [128, 8] bf16 column form via PE transposes."""
    for kl in range(8):
        nc.tensor.transpose(
            pTf[0:128, kl:kl + 1],
            csb[0:1, 128 * kl:128 * (kl + 1)],
            idtf[0:1, 0:1],
        )
    nc.vector.tensor_copy(u_sb[:, 0:8], pTf[0:128, 0:8])


def _build(num_steps: int):
    assert num_steps >= 2
    n_mid = num_steps - 2
    nc = bacc.Bacc(
        "TRN2", target_bir_lowering=False, debug=False, num_devices=NCORES
    )
    xT = nc.declare_dram_parameter("xT", [128, 8], BF16, isOutput=False)
    Wa = nc.declare_dram_parameter("Wa", [128, 8 * CP], BF16, isOutput=False)
    Wb = nc.declare_dram_parameter("Wb", [NBLK, 128, 8 * CPB], BF16, isOutput=False)
    W4 = nc.declare_dram_parameter("W4", [128, 8 * OUT_N], BF16, isOutput=False)
    identf = nc.declare_dram_parameter("identf", [1, 1], F32, isOutput=False)
    out = nc.declare_dram_parameter("out", [1, OUT_N], F32, isOutput=True)

    rs_ins = [nc.dram_tensor(f"rs{m}_in", [1, N], F32) for m in range(n_mid)]
    rs_outs = [
        nc.dram_tensor(f"rs{m}_out", [1, CP], F32) for m in range(n_mid)
    ]

    with tile.TileContext(nc) as tc:
        with (
            tc.tile_pool(name="small", bufs=1) as small,
            tc.tile_pool(name="wres", bufs=1) as wres,
            tc.tile_pool(name="ppool", bufs=1, space="PSUM") as ppool,
        ):
            xt = small.tile([128, 8], BF16, name="xt")
            nc.scalar.dma_start(out=xt[:, :], in_=xT.ap())
            w4 = small.tile([128, 8 * OUT_N], BF16, name="w4")
            nc.scalar.dma_start(out=w4[:, :], in_=W4.ap())
            idtf = small.tile([1, 1], F32, name="idtf")
            nc.scalar.dma_start(out=idtf[:, :], in_=identf.ap())

            wa = wres.tile([128, 8 * CP], BF16, name="wa")
            nc.sync.dma_start(out=wa[:, :], in_=Wa.ap())
            wk = wres.tile([128, KT * CP], BF16, name="wk")
            for b in range(NBLK):
                nc.sync.dma_start(
                    out=wk[:, b * 8 * CPB:(b + 1) * 8 * CPB],
                    in_=Wb.ap()[b],
                )

            pA = ppool.tile([128, 512], F32, name="pA")
            pP = ppool.tile([128, 1024], F32, name="pP")
            pD = ppool.tile([128, 512], F32, name="pD")
            pTf = ppool.tile([128, 8], F32, name="pTf")

            # step 1: column-style over W[0:1024, cols_d] -> local c1 chunk
            for k in range(8):
                for g in range(4):
                    nc.tensor.matmul(
                        pA[32 * g:32 * g + 1, 0:256],
                        lhsT=xt[:, k:k + 1],
                        rhs=wa[:, k * CP + 256 * g:k * CP + 256 * (g + 1)],
                        start=(k == 0),
                        stop=(k == 7),
                        tile_position=(0, 32 * g),
                    )
            c1sb = small.tile([1, CP], F32, name="c1sb")
            for g in range(4):
                eng = nc.vector.tensor_copy if g % 2 == 0 else nc.scalar.copy
                eng(out=c1sb[0:1, 256 * g:256 * (g + 1)],
                    in_=pA[32 * g:32 * g + 1, 0:256])
            u_cur = small.tile([128, 8], BF16, name="u1")
            _chunk_to_u(nc, c1sb, u_cur, pTf, idtf)

            # middle steps: row-style matvec -> ReduceScatter -> next chunk
            part = small.tile([1, N], F32, name="part")
            for m in range(n_mid):
                _row_waves(nc, pP, u_cur, wk, part)
                nc.scalar.dma_start(out=rs_ins[m].ap(), in_=part[0:1, :])
                nc.gpsimd.collective_compute(
                    "ReduceScatter", mybir.AluOpType.add, replica_groups=RG,
                    ins=[rs_ins[m].ap()], outs=[rs_outs[m].ap()],
                )
                csb = small.tile([1, CP], F32, name=f"c{m + 2}sb")
                nc.scalar.dma_start(out=csb[0:1, :], in_=rs_outs[m].ap())
                u_cur = small.tile([128, 8], BF16, name=f"u{m + 2}")
                _chunk_to_u(nc, csb, u_cur, pTf, idtf)

            # step 4: local row-slice of the tail columns (diag folded)
            for kl in range(8):
                nc.tensor.matmul(
                    pD[0:1, 0:OUT_N],
                    lhsT=u_cur[:, kl:kl + 1],
                    rhs=w4[:, OUT_N * kl:OUT_N * (kl + 1)],
                    start=(kl == 0),
                    stop=(kl == 7),
                )
            s4 = small.tile([1, OUT_N], F32, name="s4")
            nc.scalar.copy(out=s4[0:1, :], in_=pD[0:1, 0:OUT_N])
            nc.scalar.dma_start(out=out.ap(), in_=s4[0:1, :])

    nc.compile()
    return nc


def _get(num_steps: int):
    if num_steps not in _cache:
        _cache[num_steps] = _build(num_steps)
    return _cache[num_steps]


def _shard_inputs(x: np.ndarray, W: np.ndarray):
    bf = ml_dtypes.bfloat16
    xT = np.ascontiguousarray(x[0].reshape(8, 128).T).astype(bf)
    dgv = np.diagonal(W)[D0:].astype(np.float32)
    idnf = np.ones((1, 1), dtype=np.float32)
    in_maps = []
    for d in range(NCORES):
        # step-1 block: W[0:1024, cols_d], column-k-tile layout
        Wcol = W[0:IN_N, CP * d:CP * (d + 1)]
        Wa = np.ascontiguousarray(
            Wcol.reshape(8, 128, CP).transpose(1, 0, 2).reshape(128, 8 * CP)
        ).astype(bf)
        # row shard W[rows_d, :], blocked by column range, j-major inside
        T = W[CP * d:CP * (d + 1), :].reshape(8, 128, N)
        Wbk = np.ascontiguousarray(
            T.reshape(8, 128, NBLK, CPB).transpose(2, 1, 0, 3)
            .reshape(NBLK, 128, 8 * CPB)
        ).astype(bf)
        Wd4 = W[CP * d:CP * (d + 1), D0:] * dgv[None, :]
        W4 = np.ascontiguousarray(
            Wd4.reshape(8, 128, OUT_N).transpose(1, 0, 2).reshape(128, 8 * OUT_N)
        ).astype(bf)
        in_maps.append({"xT": xT, "Wa": Wa, "Wb": Wbk, "W4": W4, "identf": idnf})
    return in_maps


def _run(x, W, num_steps, trace=False):
    x = np.asarray(x, dtype=np.float32)
    W = np.asarray(W, dtype=np.float32)
    num_steps = int(num_steps)
    if num_steps == 0:
        return np.zeros(OUT_N, np.float32), None
    if num_steps == 1:
        v1d = W[0:IN_N, D0:].T.astype(np.float64) @ x[0].astype(np.float64)
        return (np.diagonal(W)[D0:] * v1d).astype(np.float32), None
    nc = _get(num_steps)
    in_maps = _shard_inputs(x, W)
    r = run_bass_kernel_spmd(
        nc, in_maps, core_ids=list(range(NCORES)), trace=trace
    )
    outv = np.sum(
        [np.asarray(r.results[d]["out"], np.float32).reshape(OUT_N)
         for d in range(NCORES)], axis=0, dtype=np.float32,
    )
    return outv, r


def kernel(x, W, num_steps) -> np.ndarray:
    outv, _ = _run(x, W, num_steps, trace=False)
    return outv


def run_traced(x, W, num_steps):
    return _run(x, W, num_steps, trace=True)


# revision 28
# speedup vs baseline: 1.2891x; 1.2891x over previous
"""Trainium2 Bass kernel for nn_AdjacencyMatrix — whole-AllGather variant.

Same column-parallel design as kernel.py, but each middle-step exchange is
ONE 2KB AllGather instead of two 1KB halves: Tile's conservative
collective-completion thresholds serialize half A behind half B anyway, so
halving only adds ~7us of serial CC-stream time.  Keeps the 4-block W
stream (HWDGE completion-sem lane fix).
"""

import ml_dtypes
import numpy as np

import concourse.bass as bass
import concourse.mybir as mybir
from concourse import bacc, tile
from concourse.bass_utils import run_bass_kernel_spmd

N = 8192
IN_N = 1024
OUT_N = 256
NCORES = 8
CP = N // NCORES
KT = N // 128
D0 = N - OUT_N
SEG = OUT_N // NCORES
NBLK = 4
KPB = KT // NBLK

F32 = mybir.dt.float32
BF16 = mybir.dt.bfloat16
RG = [list(range(NCORES))]

_cache: dict = {}


def _matvec_waves(nc, pout, u_sb, w_sb, nk, k0_tile=0, ucol0=0):
    for k in range(nk):
        wbase = (k0_tile + k) * CP
        for g in range(4):
            nc.tensor.matmul(
                pout[32 * g:32 * g + 1, 0:256],
                lhsT=u_sb[:, ucol0 + k:ucol0 + k + 1],
                rhs=w_sb[:, wbase + 256 * g:wbase + 256 * (g + 1)],
                start=(k == 0),
                stop=(k == nk - 1),
                tile_position=(0, 32 * g),
            )


def _evac(nc, s_out, pin):
    for g in range(4):
        eng = nc.vector.tensor_copy if g % 2 == 0 else nc.scalar.copy
        eng(out=s_out[0:1, 256 * g:256 * (g + 1)],
            in_=pin[32 * g:32 * g + 1, 0:256])


def _build(num_steps: int):
    assert num_steps >= 2
    n_mid = num_steps - 2
    nc = bacc.Bacc(
        "TRN2", target_bir_lowering=False, debug=False, num_devices=NCORES
    )
    xT = nc.declare_dram_parameter("xT", [128, 8], BF16, isOutput=False)
    Wb = nc.declare_dram_parameter("Wb", [NBLK, 128, KPB * CP], BF16, isOutput=False)
    W4 = nc.declare_dram_parameter("W4", [128, 8 * OUT_N], BF16, isOutput=False)
    ident = nc.declare_dram_parameter("ident", [128, 128], BF16, isOutput=False)
    out = nc.declare_dram_parameter("out", [1, OUT_N], F32, isOutput=True)

    cc_ins = [
        nc.dram_tensor(f"cc{m}_in", [1, 1024], BF16) for m in range(n_mid + 1)
    ]
    gaths = [
        nc.dram_tensor(f"G{m}", [64, 128], BF16, addr_space="Shared")
        for m in range(n_mid)
    ]

    with tile.TileContext(nc) as tc:
        with (
            tc.tile_pool(name="small", bufs=1) as small,
            tc.tile_pool(name="wres", bufs=1) as wres,
            tc.tile_pool(name="ppool", bufs=1, space="PSUM") as ppool,
        ):
            xt = small.tile([128, 8], BF16, name="xt")
            nc.scalar.dma_start(out=xt[:, :], in_=xT.ap())
            w4 = small.tile([128, 8 * OUT_N], BF16, name="w4")
            nc.scalar.dma_start(out=w4[:, :], in_=W4.ap())
            idt = small.tile([128, 128], BF16, name="idt")
            nc.scalar.dma_start(out=idt[:, :], in_=ident.ap())

            wk = wres.tile([128, KT * CP], BF16, name="wk")
            for b in range(NBLK):
                nc.sync.dma_start(
                    out=wk[:, b * KPB * CP:(b + 1) * KPB * CP],
                    in_=Wb.ap()[b],
                )

            pA = ppool.tile([128, 512], F32, name="pA")
            pB = [ppool.tile([128, 512], F32, name=f"pB{m}") for m in range(n_mid)]
            pD = ppool.tile([128, 512], F32, name="pD")
            pT = ppool.tile([128, 16], BF16, name="pT")

            _matvec_waves(nc, pA, xt, wk, nk=8, k0_tile=0, ucol0=0)
            s_cur = small.tile([1, 1024], BF16, name="s1")
            _evac(nc, s_cur, pA)

            for m in range(n_mid):
                nc.scalar.dma_start(out=cc_ins[m].ap(), in_=s_cur[0:1, :])
                nc.gpsimd.collective_compute(
                    "AllGather", mybir.AluOpType.bypass, replica_groups=RG,
                    ins=[cc_ins[m].ap()], outs=[gaths[m].ap()],
                )
                u_sb = small.tile([128, KT], BF16, name=f"u{m + 2}")
                nc.scalar.dma_start(out=u_sb[:, :], in_=gaths[m].ap(), transpose=True)
                _matvec_waves(nc, pB[m], u_sb, wk, nk=KT)
                s_cur = small.tile([1, 1024], BF16, name=f"s{m + 2}")
                _evac(nc, s_cur, pB[m])

            u4 = small.tile([128, 16], BF16, name="u4")
            for kl in range(8):
                nc.tensor.transpose(
                    pT[0:128, 2 * kl:2 * kl + 1],
                    s_cur[0:1, 128 * kl:128 * (kl + 1)],
                    idt[0:1, 0:1],
                )
            nc.vector.tensor_copy(u4[:, :], pT[0:128, 0:16])
            for kl in range(8):
                nc.tensor.matmul(
                    pD[0:1, 0:OUT_N],
                    lhsT=u4[:, 2 * kl:2 * kl + 1],
                    rhs=w4[:, OUT_N * kl:OUT_N * (kl + 1)],
                    start=(kl == 0),
                    stop=(kl == 7),
                )
            s4 = small.tile([1, OUT_N], F32, name="s4")
            nc.scalar.copy(out=s4[0:1, :], in_=pD[0:1, 0:OUT_N])
            nc.scalar.dma_start(out=out.ap(), in_=s4[0:1, :])

    nc.compile()
    return nc


def _get(num_steps: int):
    if num_steps not in _cache:
        _cache[num_steps] = _build(num_steps)
    return _cache[num_steps]


def _shard_inputs(x: np.ndarray, W: np.ndarray):
    bf = ml_dtypes.bfloat16
    xT = np.ascontiguousarray(x[0].reshape(8, 128).T).astype(bf)
    dgv = np.diagonal(W)[D0:].astype(np.float32)
    idn = np.eye(128, dtype=np.float32).astype(bf)
    in_maps = []
    for d in range(NCORES):
        Wd = W[:, CP * d:CP * (d + 1)]
        Wb = np.ascontiguousarray(
            Wd.reshape(NBLK, KPB, 128, CP).transpose(0, 2, 1, 3)
            .reshape(NBLK, 128, KPB * CP)
        ).astype(bf)
        Wd4 = W[CP * d:CP * (d + 1), D0:] * dgv[None, :]
        W4 = np.ascontiguousarray(
            Wd4.reshape(8, 128, OUT_N).transpose(1, 0, 2).reshape(128, 8 * OUT_N)
        ).astype(bf)
        in_maps.append({"xT": xT, "Wb": Wb, "W4": W4, "ident": idn})
    return in_maps


def _run(x, W, num_steps, trace=False):
    x = np.asarray(x, dtype=np.float32)
    W = np.asarray(W, dtype=np.float32)
    num_steps = int(num_steps)
    if num_steps == 0:
        return np.zeros(OUT_N, np.float32), None
    if num_steps == 1:
        v1d = W[0:IN_N, D0:].T.astype(np.float64) @ x[0].astype(np.float64)
        return (np.diagonal(W)[D0:] * v1d).astype(np.float32), None
    nc = _get(num_steps)
    in_maps = _shard_inputs(x, W)
    r = run_bass_kernel_spmd(
        nc, in_maps, core_ids=list(range(NCORES)), trace=trace
    )
    outv = np.sum(
        [np.asarray(r.results[d]["out"], np.float32).reshape(OUT_N)
         for d in range(NCORES)], axis=0, dtype=np.float32,
    )
    return outv, r


def kernel(x, W, num_steps) -> np.ndarray:
    outv, _ = _run(x, W, num_steps, trace=False)
    return outv


def run_traced(x, W, num_steps):
    return _run(x, W, num_steps, trace=True)

